# revision 10
# baseline (speedup 1.0000x reference)
"""Trainium2 Bass kernel for a 2-layer equivariant GNN message-passing network.

Strategy (8 NeuronCores, SPMD):
  - Destination-sharded: core k owns dest nodes [k*6250, (k+1)*6250), reordered
    by degree so each chunk of 128 dests has near-uniform degree.
  - Edges live on a [128 dests x S slots] grid per chunk; the segment-sum is a
    dense strided reduce on the vector engine.
  - msg = relu(u[src] + v[dst] + dist * w_d) with u = emb@mw_src,
    v = emb@mw_dst + mb.  dist is static (x never changes): the per-edge dist
    grid is precomputed on host and shipped as data.
  - u rows are fetched per-edge with dma_gather on 4 SWDGE queues. int16
    indices force two node tables (<32768 rows each); nodes are assigned to
    tables at sigma-chunk granularity and each dest's slots split into a T0
    region and a T1 region per chunk group.
  - Each layer: AllGather emb^T blocks; every core builds the full u tables
    locally (dense PE matmuls), v for its own dests; then gather + DVE grid
    math + strided reduce; then the node-update MLP on [feat, node] tiles.
  - Host does the final unpermute (free).
"""

import os
import sys

import numpy as np

for _p in ("/opt/trn_rl_repo",):
    if os.path.isdir(_p) and _p not in sys.path:
        sys.path.insert(0, _p)

import ml_dtypes  # noqa: E402

import concourse.bacc as bacc  # noqa: E402
import concourse.mybir as mybir  # noqa: E402
import concourse.tile as tile  # noqa: E402
from concourse.bass_utils import run_bass_kernel_spmd  # noqa: E402
from concourse.masks import make_identity  # noqa: E402

N = 50000
E = 800000
H = 64
CIN = 32
NCORES = 8
D_PER = 6250            # real dests per core
NT = 49                 # dest chunks of 128 per core
PER = NT * 128          # padded dests per core (6272)
NCH = NCORES * NT       # global sigma chunks (392)
ROWB = 128              # table row elements (bf16) = 256 bytes
NQ = 4                  # SWDGE queues
CAP = 96                # max slots per chunk-group (SBUF budget)
BF16 = ml_dtypes.bfloat16

LAST_RESULTS = None
DEBUG = {}


def _bf(a):
    return np.ascontiguousarray(np.asarray(a, np.float32).astype(BF16))


def _host_prep(x, h, edge_index):
    row = np.asarray(edge_index[0], np.int64)
    col = np.asarray(edge_index[1], np.int64)
    x = np.asarray(x, np.float32)
    h = np.asarray(h, np.float32)

    core_of_dest = np.minimum(col // D_PER, NCORES - 1)

    # ---- sigma: per-core degree-sorted dest order -------------------------
    orig2sigma = np.full(N, -1, np.int64)
    sigma2orig = np.full(NCORES * PER, -1, np.int64)
    per_core = []
    for k in range(NCORES):
        sel = np.nonzero(core_of_dest == k)[0]
        dloc = col[sel] - k * D_PER
        deg = np.bincount(dloc, minlength=D_PER)
        order = np.argsort(-deg, kind="stable")
        inv = np.empty(D_PER, np.int64)
        inv[order] = np.arange(D_PER)
        orig2sigma[k * D_PER + order] = k * PER + np.arange(D_PER)
        sigma2orig[k * PER + np.arange(D_PER)] = k * D_PER + order
        per_core.append((sel, dloc, inv))

    # ---- sigma-chunk -> table assignment (greedy edge balance) ------------
    src_sig = orig2sigma[row]
    src_chunk = src_sig // 128
    edges_per_chunk = np.bincount(src_chunk, minlength=NCH)
    assign = np.zeros(NCH, np.int8)
    tot = [0, 0]
    ncht = [0, 0]
    half = NCH // 2
    for c in np.argsort(-edges_per_chunk, kind="stable"):
        t = 0 if tot[0] <= tot[1] else 1
        if ncht[t] >= half:
            t = 1 - t
        assign[c] = t
        tot[t] += int(edges_per_chunk[c])
        ncht[t] += 1
    chunk_rank = np.zeros(NCH, np.int64)
    chunk_rank[assign == 0] = np.arange(ncht[0])
    chunk_rank[assign == 1] = np.arange(ncht[1])
    NROW = [ncht[0] * 128, ncht[1] * 128]
    assert NROW[0] + 1 < 32768 and NROW[1] + 1 < 32768

    # table row of a sigma idx: p-major layout row = p * ncht[t] + rank
    sig_tab = assign[src_chunk]
    sig_row = (src_sig % 128) * np.where(sig_tab == 0, ncht[0], ncht[1]) \
        + chunk_rank[src_chunk]

    # ---- per (core, chunk) split maxima -----------------------------------
    S0 = np.zeros((NCORES, NT), np.int64)
    S1 = np.zeros((NCORES, NT), np.int64)
    core_data = []
    for k in range(NCORES):
        sel, dloc, inv = per_core[k]
        j = inv[dloc]
        et = sig_tab[sel]
        k0 = np.bincount(j[et == 0], minlength=PER)
        k1 = np.bincount(j[et == 1], minlength=PER)
        S0[k] = k0.reshape(NT, 128).max(axis=1)
        S1[k] = k1.reshape(NT, 128).max(axis=1)
        core_data.append((sel, j, et))
    S0u = S0.max(axis=0).astype(int)
    S1u = S1.max(axis=0).astype(int)

    # ---- chunk groups with uniform (s0, s1), capped slot count ------------
    groups = []          # (c0, n, s0, s1)
    c = 0
    while c < NT:
        s0, s1 = S0u[c], S1u[c]
        n = 1
        while c + n < NT:
            m0 = max(s0, S0u[c + n])
            m1 = max(s1, S1u[c + n])
            if (m0 + m1) * (n + 1) > CAP:
                break
            cur = sum(S0u[c + i] + S1u[c + i] for i in range(n + 1))
            if cur > 0 and (m0 + m1) * (n + 1) > 1.12 * cur:
                break
            s0, s1, n = m0, m1, n + 1
        groups.append((c, n, s0, s1))
        c += n

    group_off = []
    off = 0
    for (c0, n, s0, s1) in groups:
        group_off.append(off)
        off += n * (s0 + s1)
    TOT = off
    E_grid = TOT * 128

    chunk_group = np.zeros(NT, np.int64)
    chunk_in_group = np.zeros(NT, np.int64)
    for gi, (c0, n, s0, s1) in enumerate(groups):
        for i in range(n):
            chunk_group[c0 + i] = gi
            chunk_in_group[c0 + i] = i
    g_n = np.array([g[1] for g in groups])
    g_s0 = np.array([g[2] for g in groups])
    g_s1 = np.array([g[3] for g in groups])
    g_off = np.array(group_off)

    # region table id per flat slot
    slot_tab = np.zeros(TOT, np.int8)
    for gi, (c0, n, s0, s1) in enumerate(groups):
        a = group_off[gi]
        slot_tab[a + n * s0:a + n * (s0 + s1)] = 1

    # ---- per-core idx + dist grids ---------------------------------------
    idx_all = np.zeros((NCORES, TOT * 128), np.int32)
    dist_all = np.zeros((NCORES, TOT * 128), np.float32)
    for k in range(NCORES):
        sel, j, et = core_data[k]
        c = j // 128
        p = j % 128
        key = j * 2 + et
        o = np.argsort(key, kind="stable")
        ks = key[o]
        cnt = np.bincount(ks, minlength=PER * 2)
        start = np.concatenate([[0], np.cumsum(cnt)[:-1]])
        slot = np.arange(ks.size) - start[ks]
        gi = chunk_group[c[o]]
        ii = chunk_in_group[c[o]]
        r = et[o]
        base = g_off[gi] + np.where(r == 0, 0, g_n[gi] * g_s0[gi])
        sr = np.where(r == 0, g_s0[gi], g_s1[gi])
        flat = base + ii * sr + slot
        pos = flat * 128 + p[o]
        grid = np.full(TOT * 128, -1, np.int64)
        grid[pos] = np.arange(len(o))
        filled = grid >= 0
        eo = grid[filled]
        idx_all[k][filled] = sig_row[sel][o][eo]
        dxa = x[row[sel][o][eo]] - x[col[sel][o][eo]]
        dist_all[k][filled] = (dxa * dxa).sum(axis=1)
        # pads -> the region table's pad row
        padtab = np.repeat(slot_tab, 128)
        padvals = np.where(padtab == 0, NROW[0], NROW[1])
        idx_all[k][~filled] = padvals[~filled]

    def wrap16(v):
        a = np.ascontiguousarray(v.reshape(-1, 16).T.astype(np.int16))
        return np.tile(a, (8, 1))

    DEBUG["pad_factor"] = E_grid * NCORES / float(E)
    DEBUG["tot_slots"] = TOT
    DEBUG["nrow"] = NROW
    DEBUG["ngroups"] = len(groups)

    in_maps = []
    for k in range(NCORES):
        s2o = sigma2orig[k * PER:(k + 1) * PER]
        valid = s2o >= 0
        hk = np.zeros((PER, CIN), np.float32)
        hk[valid] = h[s2o[valid]]
        in_maps.append({
            "hT": _bf(hk.T),
            "idx": wrap16(idx_all[k]),
            "distg": _bf(dist_all[k].reshape(TOT, 128).T),
        })

    meta = dict(groups=groups, group_off=group_off, TOT=TOT, NROW=NROW,
                assign=assign, chunk_rank=chunk_rank, ncht=ncht)
    return in_maps, sigma2orig, meta


def _weights_np(w_init, b_init, layers):
    weights = {
        "w_init": _bf(w_init),
        "b_init": np.asarray(b_init, np.float32).reshape(H, 1),
    }
    for l, (rp, mw, mb, uw, ub) in enumerate(layers):
        sl = str(l)
        weights["mu" + sl] = _bf(mw[0:H])
        weights["mv" + sl] = _bf(np.concatenate([mw[H:2 * H], mb[None, :]], 0))
        weights["wdr" + sl] = _bf(np.broadcast_to(mw[2 * H][None, :], (128, H)))
        weights["uwe" + sl] = _bf(uw[0:H])
        weights["uwa" + sl] = _bf(uw[H:2 * H])
        weights["rp" + sl] = _bf(rp)
        weights["ub" + sl] = np.asarray(ub, np.float32).reshape(H, 1)
    return weights


def _pack_weights(weights):
    layout = {}
    bcols, fcols = [], []
    bc = fc = 0
    for name, arr in sorted(weights.items()):
        r, c = arr.shape
        if arr.dtype == np.float32:
            a = np.zeros((64, c), np.float32)
            a[:r] = arr
            fcols.append(a)
            layout[name] = ("f", r, fc, c)
            fc += c
        else:
            a = np.zeros((128, c), BF16)
            a[:r] = arr
            bcols.append(a)
            layout[name] = ("b", r, bc, c)
            bc += c
    return np.concatenate(bcols, 1), np.concatenate(fcols, 1), layout


def _update_chunk(nc, psum, work, aggr, aug, out_sb, ident, wsb, sl, c, last):
    fp32 = mybir.dt.float32
    bf16 = mybir.dt.bfloat16
    AF = mybir.ActivationFunctionType
    ALU = mybir.AluOpType
    cs = slice(c * 128, (c + 1) * 128)

    pst = psum.tile([H, 128], fp32, tag="ps_t", bufs=1)
    nc.tensor.transpose(out=pst[:], in_=aggr[:], identity=ident[:])
    aggrT = work.tile([H, 128], bf16, tag="aggrT")
    nc.scalar.copy(out=aggrT[:], in_=pst[:])

    ps2 = psum.tile([H, 128], fp32, tag="ps_mlp", bufs=2)
    nc.tensor.matmul(out=ps2[:], lhsT=wsb["uwe" + sl], rhs=aug[0:H, cs],
                     start=True, stop=False)
    nc.tensor.matmul(out=ps2[:], lhsT=wsb["uwa" + sl], rhs=aggrT[:],
                     start=False, stop=True)
    psr = psum.tile([H, 128], fp32, tag="ps_res", bufs=1)
    nc.tensor.matmul(out=psr[:], lhsT=wsb["rp" + sl], rhs=aug[0:H, cs],
                     start=True, stop=True)
    tmp = work.tile([H, 128], bf16, tag="tmp")
    nc.scalar.activation(out=tmp[:], in_=ps2[:], func=AF.Relu,
                         bias=wsb["ub" + sl])
    if last:
        nc.vector.tensor_tensor(out=out_sb[:, cs], in0=tmp[:], in1=psr[:],
                                op=ALU.add)
    else:
        nc.vector.tensor_tensor(out=aug[0:H, cs], in0=tmp[:], in1=psr[:],
                                op=ALU.add)


def _build_nc(meta, wb, wf, wlayout):
    fp32 = mybir.dt.float32
    bf16 = mybir.dt.bfloat16
    i16 = mybir.dt.int16
    AF = mybir.ActivationFunctionType
    ALU = mybir.AluOpType
    X = mybir.AxisListType.X

    groups = meta["groups"]
    group_off = meta["group_off"]
    TOT = meta["TOT"]
    NROW = meta["NROW"]
    ncht = meta["ncht"]
    assign = meta["assign"]
    chunk_rank = meta["chunk_rank"]

    nc = bacc.Bacc(None, target_bir_lowering=False, num_swdge_queues=NQ)

    hT_p = nc.declare_dram_parameter("hT", [CIN, PER], bf16, isOutput=False)
    idx_p = nc.declare_dram_parameter("idx", [128, TOT * 8], i16,
                                      isOutput=False)
    distg_p = nc.declare_dram_parameter("distg", [128, TOT], bf16,
                                        isOutput=False)
    wb_p = nc.declare_dram_parameter("wb", list(wb.shape), bf16,
                                     isOutput=False)
    wf_p = nc.declare_dram_parameter("wf", list(wf.shape), fp32,
                                     isOutput=False)
    out_p = nc.declare_dram_parameter("out", [H, PER], fp32, isOutput=True)

    ownT = nc.dram_tensor("ownT", [H, PER], bf16)
    allT = nc.dram_tensor("allT", [NCORES, H, PER], bf16, addr_space="Shared")
    tabs = [nc.dram_tensor(f"tab{t}", [NROW[t] + 128, ROWB], bf16)
            for t in range(2)]
    RG = [list(range(NCORES))]

    with tile.TileContext(nc) as tc:
        with (
            tc.tile_pool(name="const", bufs=1) as const,
            tc.tile_pool(name="work", bufs=3) as work,
            tc.tile_pool(name="wavep", bufs=2) as wavep,
            tc.tile_pool(name="psum", bufs=1, space="PSUM") as psum,
        ):
            aug = const.tile([128, PER], bf16, tag="aug")
            combo = const.tile([128, NT * H], bf16, tag="combo")
            idx_sb = const.tile([128, TOT * 8], i16, tag="idx")
            distg = const.tile([128, TOT], bf16, tag="distg")
            hT_sb = const.tile([CIN, PER], bf16, tag="hT")
            out_sb = const.tile([H, PER], fp32, tag="out")
            ident = const.tile([128, 128], fp32, tag="ident")
            padrow_sb = const.tile([1, ROWB], bf16, tag="padrow")
            wb_sb = const.tile(list(wb.shape), bf16, tag="wb")
            wf_sb = const.tile(list(wf.shape), fp32, tag="wf")

            nc.sync.dma_start(out=wb_sb[:], in_=wb_p[:])
            nc.sync.dma_start(out=wf_sb[:], in_=wf_p[:])
            nc.sync.dma_start(out=idx_sb[:], in_=idx_p[:])
            nc.sync.dma_start(out=distg[:], in_=distg_p[:])
            nc.sync.dma_start(out=hT_sb[:], in_=hT_p[:])
            wsb = {}
            for name, (kind, rows, c0, cols) in wlayout.items():
                src = wb_sb if kind == "b" else wf_sb
                wsb[name] = src[0:rows, c0:c0 + cols]
            make_identity(nc, ident[:])
            nc.vector.memset(aug[64:65, :], 1.0)
            nc.vector.memset(padrow_sb[:1, :], -100000.0)
            for t in range(2):
                nc.gpsimd.dma_start(out=tabs[t][NROW[t]:NROW[t] + 1, :],
                                    in_=padrow_sb[:1, :])

            # ---- initial embedding: embT = w_init^T @ hT + b ----
            for t4 in range(0, NT, 8):
                nb = min(8, NT - t4)
                ps = psum.tile([H, 1024], fp32, tag="ps_init", bufs=1)
                for i in range(nb):
                    t = t4 + i
                    nc.tensor.matmul(out=ps[:, i * 128:(i + 1) * 128],
                                     lhsT=wsb["w_init"],
                                     rhs=hT_sb[:, t * 128:(t + 1) * 128],
                                     start=True, stop=True)
                nc.scalar.activation(out=aug[0:H, t4 * 128:(t4 + nb) * 128],
                                     in_=ps[:, 0:nb * 128], func=AF.Identity,
                                     bias=wsb["b_init"])

            for l in range(2):
                last = l == 1
                sl = str(l)
                # ---- AllGather emb^T ----
                nc.sync.dma_start(out=ownT[:], in_=aug[0:H, :])
                nc.gpsimd.collective_compute(
                    "AllGather", ALU.bypass, replica_groups=RG,
                    ins=[ownT[:]], outs=[allT[:]])

                # ---- build full u tables locally ----
                for t in range(2):
                    chunks = [c for c in range(NCH) if assign[c] == t]
                    for b0 in range(0, len(chunks), 8):
                        blk = chunks[b0:b0 + 8]
                        ps = psum.tile([128, 512], fp32, tag="ps_tab", bufs=2)
                        st = work.tile([128, 8 * ROWB], bf16, tag="stg")
                        for i, c in enumerate(blk):
                            r, cc = c // NT, c % NT
                            lhs = work.tile([H, 128], bf16, tag="lhs_emb")
                            nc.sync.dma_start(
                                out=lhs[:],
                                in_=allT[r, :, cc * 128:(cc + 1) * 128])
                            nc.tensor.matmul(out=ps[:, i * 64:(i + 1) * 64],
                                             lhsT=lhs[:], rhs=wsb["mu" + sl],
                                             start=True, stop=True)
                        nc.scalar.copy(
                            out=st[:].rearrange("p (g w) -> p g w", w=ROWB)
                                [:, 0:len(blk), 0:64],
                            in_=ps[:, 0:len(blk) * 64]
                                .rearrange("p (g w) -> p g w", w=64))
                        g0 = int(chunk_rank[blk[0]])
                        nc.sync.dma_start(
                            out=tabs[t][0:NROW[t], :]
                                .rearrange("(p g) w -> p g w", p=128)
                                [:, g0:g0 + len(blk), :],
                            in_=st[:].rearrange("p (g w) -> p g w", w=ROWB)
                                [:, 0:len(blk), :])

                # ---- v (+mb) for own dests ----
                for t4 in range(0, NT, 8):
                    nb = min(8, NT - t4)
                    ps = psum.tile([128, 512], fp32, tag="ps_tab", bufs=2)
                    for i in range(nb):
                        t = t4 + i
                        nc.tensor.matmul(
                            out=ps[:, i * 64:(i + 1) * 64],
                            lhsT=aug[0:H + 1, t * 128:(t + 1) * 128],
                            rhs=wsb["mv" + sl], start=True, stop=True)
                    nc.vector.tensor_copy(out=combo[:, t4 * 64:(t4 + nb) * 64],
                                          in_=ps[:, 0:nb * 64])

                # ---- edge stage ----
                for gi, (c0, n, s0, s1) in enumerate(groups):
                    goff = group_off[gi]
                    nslots = n * (s0 + s1)
                    if nslots == 0:
                        for i in range(n):
                            aggr = work.tile([128, H], fp32, tag="aggr")
                            nc.vector.memset(aggr[:], 0.0)
                            _update_chunk(nc, psum, work, aggr, aug, out_sb,
                                          ident, wsb, sl, c0 + i, last)
                        continue
                    wt = wavep.tile([128, CAP * ROWB], bf16, tag="wt")
                    wt3 = wt[:].rearrange("p (s w) -> p s w", w=ROWB)
                    # gathers: per region, split across NQ queues
                    for (r0, nsl, t) in ((0, n * s0, 0), (n * s0, n * s1, 1)):
                        if nsl == 0:
                            continue
                        per = -(-nsl // NQ)
                        for q in range(NQ):
                            qa = r0 + min(q * per, nsl)
                            qb = r0 + min((q + 1) * per, nsl)
                            if qb <= qa:
                                continue
                            nc.gpsimd.dma_gather(
                                wt3[:, qa:qb, :],
                                tabs[t][:, :],
                                idx_sb[:, (goff + qa) * 8:(goff + qb) * 8],
                                (qb - qa) * 128, (qb - qa) * 128, ROWB,
                                single_packet=False, queue_num=q)
                    # DVE math per region
                    tm = work.tile([128, CAP * 64], bf16, tag="tm", bufs=2)
                    for (r0, sr) in ((0, s0), (n * s0, s1)):
                        if sr == 0:
                            continue
                        u4 = wt3[:, r0:r0 + n * sr, 0:64].rearrange(
                            "p (i s) w -> p i s w", s=sr)
                        d4 = distg[:, goff + r0:goff + r0 + n * sr] \
                            .rearrange("p (i s) -> p i s", s=sr) \
                            .unsqueeze(3).to_broadcast([128, n, sr, 64])
                        c4 = combo[:, c0 * 64:(c0 + n) * 64] \
                            .rearrange("p (i w) -> p i w", w=64) \
                            .unsqueeze(2).to_broadcast([128, n, sr, 64])
                        w4 = wsb["wdr" + sl].unsqueeze(1).unsqueeze(2) \
                            .to_broadcast([128, n, sr, 64])
                        t4v = tm[:].rearrange("p (s w) -> p s w", w=64)[
                            :, 0:n * sr, :].rearrange(
                            "p (i s) w -> p i s w", s=sr)
                        nc.vector.tensor_tensor(out=u4, in0=u4, in1=c4,
                                                op=ALU.add)
                        nc.vector.tensor_tensor(out=t4v, in0=w4, in1=d4,
                                                op=ALU.mult)
                        nc.vector.tensor_tensor(out=u4, in0=u4, in1=t4v,
                                                op=ALU.add)
                        if gi % 2 == 0:
                            nc.scalar.activation(out=u4, in_=u4, func=AF.Relu)
                        else:
                            nc.vector.tensor_relu(u4, u4)
                    # strided reduce + node update per chunk
                    for i in range(n):
                        aggr = work.tile([128, H], fp32, tag="aggr")
                        have = False
                        for (r0, sr) in ((0, s0), (n * s0, s1)):
                            if sr == 0:
                                continue
                            view = wt3[:, r0 + i * sr:r0 + (i + 1) * sr, 0:64] \
                                .rearrange("p s w -> p w s")
                            if not have:
                                nc.vector.reduce_sum(out=aggr[:], in_=view,
                                                     axis=X)
                                have = True
                            else:
                                a2 = work.tile([128, H], fp32, tag="aggr2")
                                nc.vector.reduce_sum(out=a2[:], in_=view,
                                                     axis=X)
                                nc.vector.tensor_tensor(
                                    out=aggr[:], in0=aggr[:], in1=a2[:],
                                    op=ALU.add)
                        if not have:
                            nc.vector.memset(aggr[:], 0.0)
                        _update_chunk(nc, psum, work, aggr, aug, out_sb,
                                      ident, wsb, sl, c0 + i, last)

            nc.sync.dma_start(out=out_p[:], in_=out_sb[:])
    nc.compile()
    return nc


def _install_profile_hook():
    try:
        import types

        import antenv
        if "antenv.axon_hooks" not in sys.modules:
            mod = types.ModuleType("antenv.axon_hooks")
            _h = {"hook": None}
            mod.set_axon_ntff_profile_hook = \
                lambda x: _h.__setitem__("hook", x)
            mod.get_axon_ntff_profile_hook = lambda: _h["hook"]
            sys.modules["antenv.axon_hooks"] = mod
            antenv.axon_hooks = mod
            if os.path.isdir("/root/.axon_site") and \
                    "/root/.axon_site" not in sys.path:
                sys.path.insert(0, "/root/.axon_site")
            from trn_agent_boot.trn_boot import _ntff_profile_via_ctypes
            so = "/opt/axon/libaxon_pjrt.so"
            if os.path.exists(so):
                mod.set_axon_ntff_profile_hook(_ntff_profile_via_ctypes(so))
    except Exception:
        pass


def kernel(x, h, edge_index, batch, w_init, b_init,
           rp0, mw0, mb0, uw0, ub0, rp1, mw1, mb1, uw1, ub1):
    global LAST_RESULTS
    trace = bool(os.environ.get("BASS_TRACE"))
    if trace:
        _install_profile_hook()
    in_maps, sigma2orig, meta = _host_prep(x, h, edge_index)
    weights = _weights_np(w_init, b_init,
                          [(rp0, mw0, mb0, uw0, ub0),
                           (rp1, mw1, mb1, uw1, ub1)])
    wb, wf, wlayout = _pack_weights(weights)
    for m in in_maps:
        m["wb"] = wb
        m["wf"] = wf

    nc = _build_nc(meta, wb, wf, wlayout)
    res = run_bass_kernel_spmd(nc, in_maps, core_ids=list(range(NCORES)),
                               trace=trace)
    LAST_RESULTS = res

    out = np.zeros((N, H), np.float32)
    for k in range(NCORES):
        ok = np.asarray(res.results[k]["out"], np.float32)
        s2o = sigma2orig[k * PER:(k + 1) * PER]
        valid = s2o >= 0
        out[s2o[valid]] = ok[:, valid].T
    return out
